# revision 1
# baseline (speedup 1.0000x reference)
"""Trainium2 Bass kernel: per-row Euclidean projection onto
{p : 0 <= p <= PMAX, sum(p) <= BUDGET} (water-filling).

Full input raw_power (8192, 4096) f32 is sharded row-wise across 8 cores
(1024 rows each). Per core, rows live one-per-partition in 8 tiles of
[128, 4096]. The row threshold tau solving
    g(tau) = sum_i clip(x_i - tau, 0, PMAX) = BUDGET
is found per row with a safeguarded false-position (Illinois) iteration
followed by one Newton correction (the same correction the reference
applies after its 60-step bisection):

  * g-evals use the numerically-stable split
        g(tau) = R(tau) - R(tau + PMAX),   R(s) = sum_i relu(x_i - s)
    (relu sums stay small; clip-style sums at |x|~tau*N magnitude lose
    100x more precision in fp32 sequential accumulation).
  * R passes run fused+accumulated: on ACT as activation(Relu, bias=-s)
    with accum_out, on DVE as scalar_tensor_tensor((x-s) max 0) with
    accum_out. All accumulating engine paths retire 1 elem/lane/cycle
    regardless of dtype, so the early evals instead read a 4x
    column-subsampled view (stride-4 AP) at 1/4 cost; the resulting
    ~0.05 tau noise is collapsed by two full-width evals and the exact
    Newton step (full g(tau0) plus exact n_active counts).
  * The initial bracket is [0, 64] with f(64) = -BUDGET known exactly;
    the first candidate is the analytic guess tau ~= (g(0)-BUDGET)/100,
    which upper-bounds the root for this problem family and avoids a
    full row-max reduction entirely.
  * n_active counts: #(x > tau0) on DVE (is_gt + add-reduce) and
    #(x >= tau0+PMAX) on ACT via Sign accumulation
    (#pos = (sum sign + N)/2), balancing the two engines.
  * Rows already feasible (g(0) <= BUDGET) use tau = 0 == clip(x,0,PMAX).

Per-row scalar state for all 8 tiles is batched in [128, 8] tiles so each
Illinois update chain costs ~20 tiny DVE ops per iteration total. R-pass
outputs land in [128,1]-broadcast dummy tiles (only accum_out matters).
"""

import numpy as np

import concourse.bass as bass
import concourse.bacc as bacc
import concourse.mybir as mybir
from concourse.tile import TileContext
from concourse.bass_utils import run_bass_kernel_spmd

N_CORES = 8
ROWS = 8192
FD = 4096               # links per row
ROWS_PER_CORE = ROWS // N_CORES
P = 128                 # SBUF partitions
T = ROWS_PER_CORE // P  # 8 row-tiles per core
PMAX = 0.1
BUDGET = 100.0
STRIDES = [4, 4, 1, 1, 1]   # per-eval column stride: g(0), then Illinois
M_DVE = {4: 8, 1: 7}        # R2 passes on DVE per stride (rest on ACT)

F32 = mybir.dt.float32
Alu = mybir.AluOpType
Act = mybir.ActivationFunctionType
Axis = mybir.AxisListType


def _build_nc() -> bass.Bass:
    nc = bacc.Bacc("TRN2", target_bir_lowering=False)
    x_d = nc.dram_tensor("x", [ROWS_PER_CORE, FD], F32, kind="ExternalInput")
    y_d = nc.dram_tensor("y", [ROWS_PER_CORE, FD], F32, kind="ExternalOutput")
    xt = x_d[:, :].rearrange("(t p) d -> t p d", p=P)
    yt = y_d[:, :].rearrange("(t p) d -> t p d", p=P)

    with TileContext(nc) as tc:
        with (
            tc.tile_pool(name="data", bufs=1) as data,
            tc.tile_pool(name="dum", bufs=16) as dum,
            tc.tile_pool(name="st", bufs=1) as st,
        ):
            V = nc.vector
            A = nc.scalar

            xs = []
            with nc.named_scope("load"):
                for t in range(T):
                    x_tile = data.tile([P, FD], F32, tag=f"x{t}", name=f"x{t}")
                    nc.sync.dma_start(x_tile[:, :], xt[t])
                    xs.append(x_tile)

            def stile(nm, dt=F32):
                return st.tile([P, T], dt, tag=nm, name=nm)

            lo = stile("lo")
            hi = stile("hi")
            f_lo = stile("f_lo")
            f_hi = stile("f_hi")
            R1 = stile("R1")        # sum relu(x - tau) accumulators
            R2 = stile("R2")        # sum relu(x - tau - PMAX) accumulators
            C1 = stile("C1")        # count x > tau0 (DVE)
            C2 = stile("C2")        # sum sign(x - tau0 - PMAX) (ACT)
            ft = stile("ft")
            sv_i = stile("sv_i", mybir.dt.int32)
            sbar_i = stile("sbar_i", mybir.dt.int32)
            last = stile("last")
            h = stile("h")
            d = stile("dnm")
            r = stile("rcp")
            w = stile("wdt")
            tv = stile("tv")        # current candidate tau per tile-column
            tp = stile("tp")        # tau + PMAX
            ntv = stile("ntv")      # -tau (ACT bias)
            ntp = stile("ntp")      # -(tau + PMAX) (ACT bias)
            infeas = stile("infeas")
            zcol = stile("zcol")    # zeros; columns broadcast as relu floor
            negp = st.tile([P, 1], F32, tag="negp", name="negp")  # -PMAX bias

            V.memset(lo[:, :], 0.0)
            V.memset(hi[:, :], 64.0)
            V.memset(f_hi[:, :], -BUDGET)
            V.memset(last[:, :], 0.0)
            V.memset(zcol[:, :], 0.0)
            V.memset(negp[:, :], -PMAX)

            def dummy(nm):
                return dum.tile([P, 1], F32, tag="dum", name=nm)

            def r_passes(k, stride, thr_neg, thr_hi_neg, thr_hi_pos):
                """One g-eval at column stride: R1[t] = sum relu(x - thr) on
                ACT, R2[t] = sum relu(x - thr - PMAX) on DVE for t < M_DVE
                else ACT. thr_* give per-tile [P,1] APs or floats."""
                m_dve = M_DVE[stride]
                for t in range(T):
                    xv = xs[t][:, ::stride] if stride > 1 else xs[t][:, :]
                    fd = FD // stride
                    o1 = dummy(f"d{k}a{t}")
                    A.activation(
                        o1[:, :].to_broadcast([P, fd]), xv, Act.Relu,
                        bias=thr_neg(t), scale=1.0,
                        accum_out=R1[:, t : t + 1],
                    )
                    o2 = dummy(f"d{k}b{t}")
                    if t < m_dve:
                        zb = zcol[:, t : t + 1].to_broadcast([P, fd])
                        V.scalar_tensor_tensor(
                            o2[:, :].to_broadcast([P, fd]), xv,
                            thr_hi_pos(t), zb,
                            op0=Alu.subtract, op1=Alu.max,
                            accum_out=R2[:, t : t + 1],
                        )
                    else:
                        A.activation(
                            o2[:, :].to_broadcast([P, fd]), xv, Act.Relu,
                            bias=thr_hi_neg(t), scale=1.0,
                            accum_out=R2[:, t : t + 1],
                        )

            def f_from_R(dst, stride):
                # f = (R1 - R2)*stride - BUDGET
                V.tensor_sub(dst[:, :], R1[:, :], R2[:, :])
                V.tensor_scalar(dst[:, :], dst[:, :], float(stride), -BUDGET,
                                op0=Alu.mult, op1=Alu.add)

            with nc.named_scope("g0"):
                r_passes("i", STRIDES[0], lambda t: 0.0, lambda t: negp[:, :],
                         lambda t: PMAX)
                f_from_R(f_lo, STRIDES[0])
                V.tensor_scalar(infeas[:, :], f_lo[:, :], 0.0, None, op0=Alu.is_gt)

            for k, stride in enumerate(STRIDES[1:]):
                with nc.named_scope(f"iter{k}"):
                    if k == 0:
                        # analytic first candidate ~ f(0)/100 (> root for
                        # this family; harmless otherwise -- it just
                        # becomes the lo end of the bracket)
                        V.tensor_scalar(tv[:, :], f_lo[:, :], 0.01, None,
                                        op0=Alu.mult)
                    else:
                        # false-position candidate
                        V.tensor_sub(d[:, :], f_hi[:, :], f_lo[:, :])
                        V.tensor_scalar(d[:, :], d[:, :], -1e-20, None, op0=Alu.min)
                        V.reciprocal(r[:, :], d[:, :])
                        V.tensor_sub(w[:, :], hi[:, :], lo[:, :])
                        V.tensor_mul(w[:, :], w[:, :], f_hi[:, :])
                        V.tensor_mul(w[:, :], w[:, :], r[:, :])
                        V.tensor_sub(tv[:, :], hi[:, :], w[:, :])
                    V.tensor_max(tv[:, :], tv[:, :], lo[:, :])
                    V.tensor_tensor(tv[:, :], tv[:, :], hi[:, :], Alu.min)
                    V.tensor_scalar(tp[:, :], tv[:, :], PMAX, None, op0=Alu.add)
                    V.tensor_scalar(ntv[:, :], tv[:, :], -1.0, None, op0=Alu.mult)
                    V.tensor_scalar(ntp[:, :], ntv[:, :], -PMAX, None, op0=Alu.add)

                    r_passes(
                        k, stride,
                        lambda t: ntv[:, t : t + 1],
                        lambda t: ntp[:, t : t + 1],
                        lambda t: tp[:, t : t + 1],
                    )

                    f_from_R(ft, stride)
                    V.tensor_scalar(sv_i[:, :], ft[:, :], 0.0, None, op0=Alu.is_gt)
                    V.tensor_scalar(sbar_i[:, :], ft[:, :], 0.0, None, op0=Alu.is_le)
                    # Illinois halving of the stale endpoint
                    V.tensor_scalar(h[:, :], last[:, :], 0.5, 0.5, op0=Alu.mult, op1=Alu.add)
                    V.tensor_mul(f_lo[:, :], f_lo[:, :], h[:, :])
                    V.tensor_scalar(h[:, :], last[:, :], -0.5, 1.0, op0=Alu.mult, op1=Alu.add)
                    V.tensor_mul(f_hi[:, :], f_hi[:, :], h[:, :])
                    V.copy_predicated(lo[:, :], sv_i[:, :], tv[:, :])
                    V.copy_predicated(f_lo[:, :], sv_i[:, :], ft[:, :])
                    V.copy_predicated(hi[:, :], sbar_i[:, :], tv[:, :])
                    V.copy_predicated(f_hi[:, :], sbar_i[:, :], ft[:, :])
                    V.tensor_copy(last[:, :], sv_i[:, :])

            with nc.named_scope("newton"):
                # tau0 = clamped false-position candidate
                V.tensor_sub(d[:, :], f_hi[:, :], f_lo[:, :])
                V.tensor_scalar(d[:, :], d[:, :], -1e-20, None, op0=Alu.min)
                V.reciprocal(r[:, :], d[:, :])
                V.tensor_sub(w[:, :], hi[:, :], lo[:, :])
                V.tensor_mul(w[:, :], w[:, :], f_hi[:, :])
                V.tensor_mul(w[:, :], w[:, :], r[:, :])
                V.tensor_sub(tv[:, :], hi[:, :], w[:, :])
                V.tensor_max(tv[:, :], tv[:, :], lo[:, :])
                V.tensor_tensor(tv[:, :], tv[:, :], hi[:, :], Alu.min)
                V.tensor_scalar(tp[:, :], tv[:, :], PMAX, None, op0=Alu.add)
                V.tensor_scalar(ntv[:, :], tv[:, :], -1.0, None, op0=Alu.mult)
                V.tensor_scalar(ntp[:, :], ntv[:, :], -PMAX, None, op0=Alu.add)
                # exact g(tau0)
                r_passes(
                    "n", 1,
                    lambda t: ntv[:, t : t + 1],
                    lambda t: ntp[:, t : t + 1],
                    lambda t: tp[:, t : t + 1],
                )
                # exact n_active: C1 = #(x > tau0) on DVE,
                # C2 = sum sign(x - tau0 - PMAX) on ACT
                for t in range(T):
                    oc1 = dummy(f"dc1{t}")
                    V.tensor_scalar(
                        oc1[:, :].to_broadcast([P, FD]), xs[t][:, :],
                        tv[:, t : t + 1], 0.0,
                        op0=Alu.is_gt, op1=Alu.add,
                        accum_out=C1[:, t : t + 1],
                    )
                    oc2 = dummy(f"dc2{t}")
                    A.activation(
                        oc2[:, :].to_broadcast([P, FD]), xs[t][:, :], Act.Sign,
                        bias=ntp[:, t : t + 1], scale=1.0,
                        accum_out=C2[:, t : t + 1],
                    )
                # tau = tau0 + (g(tau0) - BUDGET)/n_active
                f_from_R(ft, 1)
                # n_active = C1 - (C2 + FD)/2
                V.tensor_scalar(d[:, :], C2[:, :], 0.5, float(FD) * 0.5,
                                op0=Alu.mult, op1=Alu.add)
                V.tensor_sub(d[:, :], C1[:, :], d[:, :])
                V.tensor_scalar(d[:, :], d[:, :], 1.0, None, op0=Alu.max)
                V.reciprocal(r[:, :], d[:, :])
                V.tensor_mul(ft[:, :], ft[:, :], r[:, :])
                V.tensor_add(tv[:, :], tv[:, :], ft[:, :])
                # effective tau: 0 for feasible rows
                V.tensor_mul(tv[:, :], tv[:, :], infeas[:, :])
                V.tensor_scalar(tp[:, :], tv[:, :], PMAX, None, op0=Alu.add)
                V.tensor_scalar(ntv[:, :], tv[:, :], -1.0, None, op0=Alu.mult)

            with nc.named_scope("output"):
                # out = min(max(x, tau), tau+PMAX) - tau, in place, then store
                for t in range(T):
                    V.tensor_scalar(
                        xs[t][:, :], xs[t][:, :],
                        tv[:, t : t + 1], tp[:, t : t + 1],
                        op0=Alu.max, op1=Alu.min,
                    )
                    A.activation(
                        xs[t][:, :], xs[t][:, :], Act.Identity,
                        bias=ntv[:, t : t + 1], scale=1.0,
                    )
                    nc.gpsimd.dma_start(yt[t], xs[t][:, :])

    nc.finalize()
    return nc


_NC_CACHE = None


def _get_nc():
    global _NC_CACHE
    if _NC_CACHE is None:
        _NC_CACHE = _build_nc()
    return _NC_CACHE


def run(raw_power: np.ndarray, trace: bool = False):
    """Shard, run on 8 cores, gather. Returns (output, BassKernelResults)."""
    assert raw_power.shape == (ROWS, FD), raw_power.shape
    x = np.ascontiguousarray(raw_power, dtype=np.float32)
    shards = np.split(x, N_CORES, axis=0)
    nc = _get_nc()
    res = run_bass_kernel_spmd(
        nc,
        [{"x": s} for s in shards],
        core_ids=list(range(N_CORES)),
        trace=trace,
    )
    out = np.concatenate([r["y"] for r in res.results], axis=0)
    return out, res


def kernel(raw_power: np.ndarray) -> np.ndarray:
    out, _ = run(raw_power, trace=False)
    return out

